# revision 8
# baseline (speedup 1.0000x reference)
"""Causal depthwise conv1d (B=8, S=4096, H=2048, KS=4) on 8 trn2 NeuronCores.

Strategy:
  - Shard batch across the 8 cores (one batch element each, no halo needed).
  - bf16 on the wire: host casts x to bf16 (and the result back to f32), so
    each core moves 16 MiB in + 16 MiB out instead of 32+32 — the kernel is
    DMA-bound, so halving bytes halves the roofline. bf16 rounding keeps the
    end-to-end rel err ~5e-3, well inside the 2e-2 gate.
  - Host-side transpose each batch element to (H, S): channels on SBUF
    partitions, sequence contiguous on the free axis. Conv shifts become
    free-dim AP offsets.
  - Engine split, per 2048-col half-block against the ~2.9us DMA budget:
      PE  : taps w0,w1,w2 as per-channel diagonal matmuls (bf16, ~218ns per
            512-col matmul with overlapped LDWEIGHTS)          ~2.6us
      ACT : t = w3*x + bias (per-partition scale/bias)         ~2.0us
      DVE : t += psum  (tensor_tensor, PSUM operand, 1x)       ~2.4us
  - PSUM in 1024-col (2-bank) tiles, bufs=4: a quarter's matmuls only wait
    on the TT four quarters back, keeping ~6us of WAR slack.
  - Ring hygiene (the previous revision lost 1.9us/chunk to DMA-semaphore
    reuse waits serializing the sync ring): loads are full-block 4096-col
    DMAs on the sync ring (half the issue traffic, 8KB descriptors); stores
    and the PAD memsets live on the otherwise-idle gpsimd SWDGE queue; the
    scalar ring only runs the ACT products.
"""

import numpy as np

B, S, H, KS = 8, 4096, 2048, 4
NCORES = 8
PB = 128            # SBUF partitions
HB = H // PB        # 16 channel blocks per core
PAD = 4             # left zero-pad columns in the x tile (3 used + 1 align)
HW_ = 2048          # half-block width (ACT / store granularity)
QW = 1024           # quarter width (PSUM tile = 2 banks)
BANK = 512          # PSUM bank width in f32 elements
NPE = 3             # taps computed on PE (w0, w1, w2); w3 + bias on ACT

RUN_KWARGS = {}
LAST_RESULTS = []

_cached = {}


def _build():
    import concourse.bacc as bacc
    import concourse.mybir as mybir
    import concourse.tile as tile

    f32 = mybir.dt.float32
    bf16 = mybir.dt.bfloat16
    Alu = mybir.AluOpType
    Act = mybir.ActivationFunctionType

    nc = bacc.Bacc(
        "TRN2",
        target_bir_lowering=False,
        debug=False,
        num_devices=NCORES,
    )
    xT = nc.dram_tensor("xT", [H, S], bf16, kind="ExternalInput")
    wp = nc.dram_tensor("wp", [PB, HB * 5], f32, kind="ExternalInput")
    wd = nc.dram_tensor("wd", [PB, HB * NPE * PB], bf16, kind="ExternalInput")
    yT = nc.dram_tensor("yT", [H, S], bf16, kind="ExternalOutput")

    with tile.TileContext(nc) as tc:
        with tc.tile_pool(name="wpool", bufs=1) as wpool, \
             tc.tile_pool(name="xpool", bufs=5) as xpool, \
             tc.tile_pool(name="data", bufs=3) as pool, \
             tc.tile_pool(name="ppool", bufs=4, space="PSUM") as ppool:
            wsb = wpool.tile([PB, HB * 5], f32)
            wdb = wpool.tile([PB, HB * NPE * PB], bf16)
            nc.scalar.dma_start(wsb[:], wp[:])
            nc.scalar.dma_start(wdb[:], wd[:])
            # Tiny no-dep ACTIVATE so the ACT table load overlaps the first
            # x DMA instead of serializing in front of the first product.
            warm = wpool.tile([PB, 2], bf16)
            nc.vector.memset(warm[:], 0.0)
            nc.scalar.activation(warm[:], warm[:], Act.Identity, bias=0.0,
                                 scale=1.0)

            xts = {}
            ts = {}       # hb -> full-block t tile
            pend = []     # [(hb, t)] stores not yet emitted

            for hb in range(HB + 1):
                if hb < HB:
                    rows = slice(hb * PB, (hb + 1) * PB)
                    xt = xpool.tile([PB, PAD + S], bf16)
                    xts[hb] = xt
                    nc.gpsimd.memset(xt[:, 0:PAD], 0.0)
                    nc.sync.dma_start(xt[:, PAD:PAD + S], xT[rows, :])
                    c = hb * 5
                    w3 = wsb[:, c + 3:c + 4]
                    bb = wsb[:, c + 4:c + 5]
                    t = pool.tile([PB, S], bf16, tag="t", bufs=3)
                    ts[hb] = t
                    for half in range(S // HW_):
                        s0 = half * HW_
                        base = PAD + s0
                        # ACT product for the half-block, before the matmuls
                        # so it starts as soon as the load lands
                        nc.scalar.activation(t[:, s0:s0 + HW_],
                                             xt[:, base:base + HW_],
                                             Act.Identity, bias=bb, scale=w3)
                        for q in range(HW_ // QW):
                            qbase = base + q * QW
                            ps = ppool.tile([PB, QW], f32)
                            for k in range(NPE):
                                dcol = (hb * NPE + k) * PB
                                dw = wdb[:, dcol:dcol + PB]
                                shift = qbase - (NPE - k)  # k=0->s-3..k=2->s-1
                                for b in range(QW // BANK):
                                    nc.tensor.matmul(
                                        ps[:, b * BANK:(b + 1) * BANK],
                                        dw,
                                        xt[:, shift + b * BANK:
                                               shift + (b + 1) * BANK],
                                        start=(k == 0), stop=(k == NPE - 1),
                                        skip_group_check=True)
                            # DVE: t[quarter] += psum
                            qs = s0 + q * QW
                            nc.vector.tensor_tensor(
                                t[:, qs:qs + QW],
                                t[:, qs:qs + QW], ps[:], op=Alu.add)
                        if half == 0 and pend:
                            # previous block's store, emitted mid-block so the
                            # sync ring interleaves loads and ready stores
                            phb, pt = pend.pop(0)
                            prow = slice(phb * PB, (phb + 1) * PB)
                            nc.sync.dma_start(yT[prow, :], pt[:])
                    pend.append((hb, t))
                else:
                    for phb, pt in pend:
                        prow = slice(phb * PB, (phb + 1) * PB)
                        nc.sync.dma_start(yT[prow, :], pt[:])
    nc.compile()
    return nc


def get_nc():
    if "nc" not in _cached:
        _cached["nc"] = _build()
    return _cached["nc"]


def pack_weights(weight, bias):
    wp = np.empty((PB, HB * 5), dtype=np.float32)
    for hb in range(HB):
        sl = slice(hb * PB, (hb + 1) * PB)
        for k in range(KS):
            wp[:, hb * 5 + k] = weight[k, sl]
        wp[:, hb * 5 + 4] = bias[sl]
    return wp


def pack_diag(weight):
    """Per-block diagonal matrices for taps w0..w2, bf16, [PB, HB*NPE*PB]."""
    import ml_dtypes
    wd = np.zeros((PB, HB * NPE * PB), dtype=ml_dtypes.bfloat16)
    idx = np.arange(PB)
    for hb in range(HB):
        for k in range(NPE):
            col = (hb * NPE + k) * PB
            wd[idx, col + idx] = weight[k, hb * PB + idx].astype(
                ml_dtypes.bfloat16)
    return wd


def kernel(x, weight, bias):
    import ml_dtypes
    from concourse.bass_utils import run_bass_kernel_spmd

    x = np.asarray(x, dtype=np.float32)
    weight = np.asarray(weight, dtype=np.float32)
    bias = np.asarray(bias, dtype=np.float32)
    assert x.shape == (B, S, H), x.shape
    assert weight.shape == (KS, H), weight.shape
    assert bias.shape == (H,), bias.shape

    nc = get_nc()
    wp = pack_weights(weight, bias)
    wd = pack_diag(weight)
    xT = x.transpose(0, 2, 1).astype(ml_dtypes.bfloat16)

    in_maps = [{"xT": xT[i], "wp": wp, "wd": wd} for i in range(NCORES)]
    try:
        res = run_bass_kernel_spmd(nc, in_maps, core_ids=list(range(NCORES)),
                                   **RUN_KWARGS)
    except Exception:
        res = run_bass_kernel_spmd(nc, in_maps, core_ids=list(range(NCORES)),
                                   **RUN_KWARGS)
    LAST_RESULTS.clear()
    LAST_RESULTS.append(res)
    y = np.stack([res.results[i]["yT"] for i in range(NCORES)])
    return y.transpose(0, 2, 1).astype(np.float32)


# revision 9
# speedup vs baseline: 1.1124x; 1.1124x over previous
"""Causal depthwise conv1d (B=8, S=4096, H=2048, KS=4) on 8 trn2 NeuronCores.

Strategy:
  - Shard batch across the 8 cores (one batch element each, no halo needed).
  - bf16 on the wire: host casts x to bf16 (and the result back to f32), so
    each core moves 16 MiB in + 16 MiB out instead of 32+32 — the kernel is
    DMA-bound, so halving bytes halves the roofline. bf16 rounding keeps the
    end-to-end rel err ~5e-3, well inside the 2e-2 gate.
  - Host-side transpose each batch element to (H, S): channels on SBUF
    partitions, sequence contiguous on the free axis. Conv shifts become
    free-dim AP offsets.
  - Engine split, per 2048-col half-block against the ~2.9us DMA budget:
      PE  : taps w0,w1,w2 as per-channel diagonal matmuls (bf16, ~218ns per
            512-col matmul with overlapped LDWEIGHTS)            ~2.7us
      ACT : e = psum + bias (PSUM->SBUF bf16 extraction)         ~2.1us
      DVE : t3 = w3*x (tensor_scalar, 4x) ; y = t3 + e (TT, 2x)  ~1.9us
    DVE never touches PSUM, so its ops run in fast DVE modes and the PSUM
    write-after-read recycling is resolved by ACT, one half-block behind PE.
  - Ring hygiene (earlier revisions lost 1-3us/chunk to DMA-semaphore reuse
    waits blocking rings): full-block 4096-col loads on the sync ring (8KB
    descriptors), full-block stores + PAD memsets on the gpsimd SWDGE queue,
    scalar ring = ACT only, and the merge TT is emitted one half-block late
    so no ring instruction waits at the head of its queue.
"""

import numpy as np

B, S, H, KS = 8, 4096, 2048, 4
NCORES = 8
PB = 128            # SBUF partitions
HB = H // PB        # 16 channel blocks per core
PAD = 4             # left zero-pad columns in the x tile (3 used + 1 align)
HW_ = 2048          # half-block width (PSUM tile = 4 banks)
BANK = 512          # PSUM bank width in f32 elements
NPE = 3             # taps computed on PE (w0, w1, w2); w3 on DVE

RUN_KWARGS = {}
LAST_RESULTS = []

_cached = {}


def _build():
    import concourse.bacc as bacc
    import concourse.mybir as mybir
    import concourse.tile as tile

    f32 = mybir.dt.float32
    bf16 = mybir.dt.bfloat16
    Alu = mybir.AluOpType
    Act = mybir.ActivationFunctionType

    nc = bacc.Bacc(
        "TRN2",
        target_bir_lowering=False,
        debug=False,
        num_devices=NCORES,
    )
    xT = nc.dram_tensor("xT", [H, S], bf16, kind="ExternalInput")
    wp = nc.dram_tensor("wp", [PB, HB * 5], f32, kind="ExternalInput")
    wd = nc.dram_tensor("wd", [PB, HB * NPE * PB], bf16, kind="ExternalInput")
    yT = nc.dram_tensor("yT", [H, S], bf16, kind="ExternalOutput")

    with tile.TileContext(nc) as tc:
        with tc.tile_pool(name="wpool", bufs=1) as wpool, \
             tc.tile_pool(name="xpool", bufs=5) as xpool, \
             tc.tile_pool(name="ypool", bufs=3) as ypool, \
             tc.tile_pool(name="data", bufs=4) as pool, \
             tc.tile_pool(name="ppool", bufs=2, space="PSUM") as ppool:
            wsb = wpool.tile([PB, HB * 5], f32)
            wdb = wpool.tile([PB, HB * NPE * PB], bf16)
            nc.scalar.dma_start(wsb[:], wp[:])
            nc.scalar.dma_start(wdb[:], wd[:])
            # Tiny no-dep ACTIVATE so the ACT table load overlaps the first
            # x DMA instead of serializing in front of the first extraction.
            warm = wpool.tile([PB, 2], bf16)
            nc.vector.memset(warm[:], 0.0)
            nc.scalar.activation(warm[:], warm[:], Act.Identity, bias=0.0,
                                 scale=1.0)

            xts = {}
            pend_tt = []      # [(y, t3, e, s0)] merge TTs one half behind
            pend_store = []   # [(hb, y)] stores ~a block behind

            for hb in range(HB + 1):
                if hb < HB:
                    rows = slice(hb * PB, (hb + 1) * PB)
                    xt = xpool.tile([PB, PAD + S], bf16)
                    xts[hb] = xt
                    nc.gpsimd.memset(xt[:, 0:PAD], 0.0)
                    if hb == 0:
                        # first block in halves so compute starts ~1.5us in
                        nc.sync.dma_start(xt[:, PAD:PAD + HW_],
                                          xT[rows, 0:HW_])
                        nc.sync.dma_start(xt[:, PAD + HW_:PAD + S],
                                          xT[rows, HW_:S])
                    else:
                        nc.sync.dma_start(xt[:, PAD:PAD + S], xT[rows, :])
                    c = hb * 5
                    w3 = wsb[:, c + 3:c + 4]
                    bb = wsb[:, c + 4:c + 5]
                    y = ypool.tile([PB, S], bf16)
                    for half in range(S // HW_):
                        s0 = half * HW_
                        base = PAD + s0
                        # DVE product (4x mode) while PE chews the matmuls
                        t3 = pool.tile([PB, HW_], bf16, tag="t3", bufs=4)
                        nc.vector.tensor_scalar(t3[:], xt[:, base:base + HW_],
                                                w3, None, op0=Alu.mult)
                        ps = ppool.tile([PB, HW_], f32)
                        for k in range(NPE):
                            dcol = (hb * NPE + k) * PB
                            dw = wdb[:, dcol:dcol + PB]
                            shift = base - (NPE - k)  # k=0 -> s-3 .. k=2 -> s-1
                            for b in range(HW_ // BANK):
                                nc.tensor.matmul(
                                    ps[:, b * BANK:(b + 1) * BANK],
                                    dw,
                                    xt[:, shift + b * BANK:
                                           shift + (b + 1) * BANK],
                                    start=(k == 0), stop=(k == NPE - 1),
                                    skip_group_check=True)
                        # ACT: e = psum + bias (f32 PSUM -> bf16 SBUF)
                        e = pool.tile([PB, HW_], bf16, tag="e", bufs=3)
                        nc.scalar.activation(e[:], ps[:], Act.Identity,
                                             bias=bb, scale=1.0)
                        # merge TT one half-block late: never head-blocks DVE
                        if pend_tt:
                            py, pt3, pe_, ps0 = pend_tt.pop(0)
                            nc.vector.tensor_tensor(py[:, ps0:ps0 + HW_],
                                                    pt3[:], pe_[:], op=Alu.add)
                        pend_tt.append((y, t3, e, s0))
                        if half == 1 and pend_store:
                            phb, py = pend_store.pop(0)
                            prow = slice(phb * PB, (phb + 1) * PB)
                            nc.gpsimd.dma_start(yT[prow, :], py[:])
                    pend_store.append((hb, y))
                else:
                    for py, pt3, pe_, ps0 in pend_tt:
                        nc.vector.tensor_tensor(py[:, ps0:ps0 + HW_],
                                                pt3[:], pe_[:], op=Alu.add)
                    pend_tt.clear()
                    for phb, py in pend_store:
                        prow = slice(phb * PB, (phb + 1) * PB)
                        nc.gpsimd.dma_start(yT[prow, :], py[:])
    nc.compile()
    return nc


def get_nc():
    if "nc" not in _cached:
        _cached["nc"] = _build()
    return _cached["nc"]


def pack_weights(weight, bias):
    wp = np.empty((PB, HB * 5), dtype=np.float32)
    for hb in range(HB):
        sl = slice(hb * PB, (hb + 1) * PB)
        for k in range(KS):
            wp[:, hb * 5 + k] = weight[k, sl]
        wp[:, hb * 5 + 4] = bias[sl]
    return wp


def pack_diag(weight):
    """Per-block diagonal matrices for taps w0..w2, bf16, [PB, HB*NPE*PB]."""
    import ml_dtypes
    wd = np.zeros((PB, HB * NPE * PB), dtype=ml_dtypes.bfloat16)
    idx = np.arange(PB)
    for hb in range(HB):
        for k in range(NPE):
            col = (hb * NPE + k) * PB
            wd[idx, col + idx] = weight[k, hb * PB + idx].astype(
                ml_dtypes.bfloat16)
    return wd


def kernel(x, weight, bias):
    import ml_dtypes
    from concourse.bass_utils import run_bass_kernel_spmd

    x = np.asarray(x, dtype=np.float32)
    weight = np.asarray(weight, dtype=np.float32)
    bias = np.asarray(bias, dtype=np.float32)
    assert x.shape == (B, S, H), x.shape
    assert weight.shape == (KS, H), weight.shape
    assert bias.shape == (H,), bias.shape

    nc = get_nc()
    wp = pack_weights(weight, bias)
    wd = pack_diag(weight)
    xT = x.transpose(0, 2, 1).astype(ml_dtypes.bfloat16)

    in_maps = [{"xT": xT[i], "wp": wp, "wd": wd} for i in range(NCORES)]
    try:
        res = run_bass_kernel_spmd(nc, in_maps, core_ids=list(range(NCORES)),
                                   **RUN_KWARGS)
    except Exception:
        res = run_bass_kernel_spmd(nc, in_maps, core_ids=list(range(NCORES)),
                                   **RUN_KWARGS)
    LAST_RESULTS.clear()
    LAST_RESULTS.append(res)
    y = np.stack([res.results[i]["yT"] for i in range(NCORES)])
    return y.transpose(0, 2, 1).astype(np.float32)
